# revision 21
# baseline (speedup 1.0000x reference)
"""Trainium2 Bass kernel for nn_Attention_28020366639391 (sparse attention), v3.

Math (per batch element b, reference semantics):
    q/k/v = x @ W{q,k,v} + b, 12 heads of 64; scores = q k^T / 8
    rows >= 512 zeroed pre-softmax -> those ctx rows = mean_k(v)
    out = concat_heads(ctx) @ Wo + bo

Sharding: data-parallel on batch, 8 elements -> 8 cores, no collectives.

Per-core dataflow (v3 = v2 + latency/overhead work):
  Host prep: xT in fp8 hi/lo ([128, 6, 1024], partition = d_in%128), weights
  pre-scaled by 64 then split fp8 hi/lo; Wq/Wk relaid c-chunk-major
  ([128, 6(d_out chunk), 6(d_in chunk), 128]) so startup DMAs can stream
  per-output-chunk pieces; Wo in fp8 hi/lo (also x64).
  - QKV projections: fp8 DoubleRow, 3 products (hh + hl + lh) ~ bf16-accurate
    at 0.5 cycles/row. QT/KT evac to fp16 [d_out, q]; V evac to fp16
    Vaug [keys, head*65] with a 1.0 column per head (softmax denominator;
    the x64 V scale then rides into sbctx, cancelled at the final evac).
  - scores (per head, per 256-key pair): fp16 matmuls into a paired PSUM
    tile [128, 2, 512]; ONE exp op per pair ([128, 1024]) via ACT (exact)
    or DVE/Pool (Schraudolph int16->fp16 bit trick).
  - ctx (layout B, kc-major): out[q,65] += e_slice^T @ Vaug_h per key chunk.
  - normalize fused into PSUM evac: ctx * recip(denom) -> sbctx [q, d] fp16
    (values x64 because the denom column is 1.0 while v carries x64).
  - PE-transpose sbctx -> ptT; evac splits into ctxT fp8 hi/lo; out-proj is
    fp8 DoubleRow 3-product vs Wo8 hi/lo; final evac scales by 1/4096.
  - tail rows 512:1024 = broadcast of (mean_k v) @ Wo; mean_k v from a
    ones^T @ Vaug PE reduction, split hi/lo, DR 3-product vs Wo8. Tail
    instruction chains are interleaved with late score emissions so the
    PE never stalls on the serial vector->PE->vector latency.
  DMAs are spread across the SP/ACT/Pool DGE queues, demand-ordered, with
  the first-needed pieces (x[:, :256], Wq chunk 0) split fine so the first
  matmul can start ~2.5us earlier than with monolithic transfers.
"""

import numpy as np

import concourse.bass as bass
import concourse.mybir as mybir
import concourse.tile as tile
from concourse import bacc
from concourse.bass_utils import run_bass_kernel_spmd
from concourse.masks import make_identity

B, S, D, H, DH = 8, 1024, 768, 12, 64
SH = 512            # active query rows
DC = D // 128       # 6 chunks of model dim
SC = S // 128       # 8 chunks of sequence dim
NCORES = 8
WS = 64.0           # host-side weight scaling (fp8 lo-residual range fix)
FP = mybir.dt.float32
F16 = mybir.dt.float16
F8 = mybir.dt.float8e4
I16 = mybir.dt.int16
AF = mybir.ActivationFunctionType
ALU = mybir.AluOpType
DR = mybir.MatmulPerfMode.DoubleRow

LOG2E = 1.4426950408889634
SCORE_SCALE = 0.125 / (WS * WS)      # folded 1/64^2 weight scaling
SCH_A = SCORE_SCALE * LOG2E * 1024.0 # schraudolph int16 multiplier
SCH_B = 15.0 * 1024.0 - 44.0         # schraudolph bias (tuned C=44, floor conv)
OUT_DESCALE = 1.0 / (WS * WS)        # ctx(x64) @ Wo(x64) -> /4096 at evac

# exp engine per (head, kc-pair): a=ACT(exact exp), d=DVE, p=Pool (schraudolph)
EXP_SCHED = "adadadadadad"
# normalize/evac engine per head (ACT/Pool; DVE handles recips)
NRM_SCHED = "ap"
# V-evac engine per sequence chunk
VEV_SCHED = "dadaadda"


def _mm_hilo(nc, out, lhs_hi, lhs_lo, rhs_hi, rhs_lo, first, last,
             lo_last="lhs"):
    """3-product hi/lo fp8 DoubleRow accumulation into one PSUM group.

    lo_last picks which lo-product goes last, so the operand that arrives
    last on the serial DMA stream doesn't stall the group start."""
    nc.tensor.matmul(out, lhs_hi, rhs_hi, start=first, stop=False, perf_mode=DR)
    if lo_last == "lhs":
        nc.tensor.matmul(out, lhs_hi, rhs_lo, start=False, stop=False,
                         perf_mode=DR)
        nc.tensor.matmul(out, lhs_lo, rhs_hi, start=False, stop=last,
                         perf_mode=DR)
    else:
        nc.tensor.matmul(out, lhs_lo, rhs_hi, start=False, stop=False,
                         perf_mode=DR)
        nc.tensor.matmul(out, lhs_hi, rhs_lo, start=False, stop=last,
                         perf_mode=DR)


PHASES = []


def _body(tc, out, t_in, with_bias=False):
    nc = tc.nc
    from contextlib import ExitStack

    def mark(nm):
        PHASES.append((nm, len(list(nc.all_instructions()))))

    with ExitStack() as ctx:
        ctx.enter_context(
            nc.allow_low_precision(reason="fp8 hi/lo + fp16 pipeline by design")
        )
        constp = ctx.enter_context(tc.tile_pool(name="const", bufs=1))
        wp = ctx.enter_context(tc.tile_pool(name="wp", bufs=1))
        qkp = ctx.enter_context(tc.tile_pool(name="qk", bufs=1))
        P = {}  # phase-scoped PSUM pools, resolved at call time

        # ---------------- DMA inputs (parallel queues, ordered by need) ------
        wt = {}
        for nm in ("xh8", "xl8"):
            wt[nm] = wp.tile([128, DC, S], F8, tag=nm, name=nm)
        for nm in ("wqh", "wql", "wkh", "wkl"):
            # c-chunk-major relayout: [128, c(d_out/128), kp(d_in/128), 128]
            wt[nm] = wp.tile([128, DC, DC, 128], F8, tag=nm, name=nm)
        for nm in ("wvh", "wvl", "woh", "wol"):
            wt[nm] = wp.tile([128, DC, D], F8, tag=nm, name=nm)
        # demand-ordered DMA list; queues: SP / ACT HWDGE + Pool SWDGE run
        # their desc-gens in parallel, transfers share the HBM channel.
        dmas = [
            # first QT chunk needs x[:, :256] + Wq chunk 0 only
            (wt["xh8"][:, :, 0:256], t_in["xh8"][:, :, 0:256]),
            (wt["wqh"][:, 0:2], t_in["wqh"][:, 0:2]),
            (wt["xl8"][:, :, 0:256], t_in["xl8"][:, :, 0:256]),
            (wt["wql"][:, 0:2], t_in["wql"][:, 0:2]),
            (wt["xh8"][:, :, 256:512], t_in["xh8"][:, :, 256:512]),
            (wt["xl8"][:, :, 256:512], t_in["xl8"][:, :, 256:512]),
            (wt["wqh"][:, 2:6], t_in["wqh"][:, 2:6]),
            (wt["wql"][:, 2:6], t_in["wql"][:, 2:6]),
            (wt["wkh"][:, 0:3], t_in["wkh"][:, 0:3]),
            (wt["wkl"][:, 0:3], t_in["wkl"][:, 0:3]),
            (wt["wkh"][:, 3:6], t_in["wkh"][:, 3:6]),
            (wt["wkl"][:, 3:6], t_in["wkl"][:, 3:6]),
            (wt["xh8"][:, :, 512:1024], t_in["xh8"][:, :, 512:1024]),
            (wt["xl8"][:, :, 512:1024], t_in["xl8"][:, :, 512:1024]),
            (wt["wvh"][:], t_in["wvh"][...]),
            (wt["wvl"][:], t_in["wvl"][...]),
            (wt["woh"][:], t_in["woh"][...]),
            (wt["wol"][:], t_in["wol"][...]),
        ]
        qs = [nc.sync, nc.scalar, nc.gpsimd]
        for i, (dst, src_) in enumerate(dmas):
            qs[i % 3].dma_start(out=dst, in_=src_)
        xh8, xl8 = wt["xh8"], wt["xl8"]

        # ---------------- constants ----------------
        ident = constp.tile([128, 128], F16, tag="ident")
        identf = constp.tile([128, 128], FP, tag="identf")
        make_identity(nc, identf[:])
        nc.gpsimd.tensor_copy(ident[:], identf[:])
        ones16 = constp.tile([128, 1], F16, tag="ones16")
        nc.gpsimd.memset(ones16[:], 1.0)

        QT = qkp.tile([128, DC, SH], F16, tag="QT")
        KT = qkp.tile([128, DC, S], F16, tag="KT")
        Vaug = qkp.tile([128, SC, H * 65], F16, tag="Vaug")
        # denominator columns: 1.0 (v carries x64; net sbctx = 64*ctx_true)
        nc.gpsimd.memset(
            Vaug[:].rearrange("p s (h e) -> p s h e", h=H)[:, :, :, 64:65], 1.0
        )

        if with_bias:
            b_sb = {}
            for nm in ("bq", "bk", "bv"):
                t = constp.tile([128, DC], FP, tag=f"b_{nm}", name=f"b_{nm}")
                for c in range(DC):
                    nc.sync.dma_start(
                        out=t[:, c : c + 1], in_=t_in[nm][c * 128 : (c + 1) * 128, None]
                    )
                b_sb[nm] = t
            for nm in ("bq", "bk", "bv"):  # match the x64-scaled q/k/v outputs
                nc.vector.tensor_scalar_mul(b_sb[nm][:], b_sb[nm][:], WS)
            bo_row = constp.tile([1, D], F16, tag="bo_row")
            bo_f = constp.tile([1, D], FP, tag="bo_f")
            bvr_f = constp.tile([1, D], FP, tag="bvr_f")
            nc.sync.dma_start(out=bvr_f[:], in_=t_in["bv"][None, :])
            nc.sync.dma_start(out=bo_f[:], in_=t_in["bo"][None, :])
            # po carries x4096; pre-scale bo to survive the 1/4096 evac
            nc.vector.tensor_scalar_mul(bo_f[:], bo_f[:], WS * WS)
            nc.vector.tensor_copy(bo_row[:], bo_f[:])
            ones1 = constp.tile([1, 128], F16, tag="ones1")
            nc.vector.memset(ones1[:], 1.0)

        # ---------------- QKV projections (fp8 DoubleRow hi/lo) --------------
        def qt_chunk(c, splits):
            pq = P["ppq"].tile([128, SH], FP, tag="pp")
            for n0, nw in splits:
                for kp in range(DC // 2):
                    _mm_hilo(
                        nc, pq[:, n0 : n0 + nw],
                        wt["wqh"][:, c, 2 * kp : 2 * kp + 2, :],
                        wt["wql"][:, c, 2 * kp : 2 * kp + 2, :],
                        xh8[:, 2 * kp : 2 * kp + 2, n0 : n0 + nw],
                        xl8[:, 2 * kp : 2 * kp + 2, n0 : n0 + nw],
                        first=(kp == 0), last=(kp == DC // 2 - 1),
                    )
            if with_bias:
                nc.scalar.activation(
                    QT[:, c, :], pq[:], AF.Identity, bias=b_sb["bq"][:, c : c + 1]
                )
            elif c % 2 == 0:
                nc.scalar.copy(QT[:, c, :], pq[:])
            else:
                nc.vector.tensor_copy(QT[:, c, :], pq[:])

        def kt_chunk(c, sg):
            pk = P["ppk"].tile([128, 512], FP, tag="ppk")
            for kp in range(DC // 2):
                _mm_hilo(
                    nc, pk[:],
                    wt["wkh"][:, c, 2 * kp : 2 * kp + 2, :],
                    wt["wkl"][:, c, 2 * kp : 2 * kp + 2, :],
                    xh8[:, 2 * kp : 2 * kp + 2, sg * 512 : sg * 512 + 512],
                    xl8[:, 2 * kp : 2 * kp + 2, sg * 512 : sg * 512 + 512],
                    first=(kp == 0), last=(kp == DC // 2 - 1),
                )
            dst = KT[:, c, sg * 512 : sg * 512 + 512]
            if with_bias:
                nc.vector.tensor_scalar(dst, pk[:], b_sb["bk"][:, c : c + 1],
                                        None, ALU.add)
            elif (c + sg) % 2 == 0:
                nc.vector.tensor_copy(dst, pk[:])
            else:
                nc.scalar.copy(dst, pk[:])

        def v_chunk(sc):
            pv = P["ppv"].tile([128, D], FP, tag="ppv",
                               padded_shape=[128, 1024])
            for n0, nw in ((0, 512), (512, 256)):
                for kp in range(DC // 2):
                    _mm_hilo(
                        nc, pv[:, n0 : n0 + nw],
                        xh8[:, 2 * kp : 2 * kp + 2, sc * 128 : (sc + 1) * 128],
                        xl8[:, 2 * kp : 2 * kp + 2, sc * 128 : (sc + 1) * 128],
                        wt["wvh"][:, 2 * kp : 2 * kp + 2, n0 : n0 + nw],
                        wt["wvl"][:, 2 * kp : 2 * kp + 2, n0 : n0 + nw],
                        first=(kp == 0), last=(kp == DC // 2 - 1),
                        lo_last="rhs",
                    )
            dst = Vaug[:].rearrange("p s (h e) -> p s h e", h=H)[:, sc, :, 0:64]
            src = pv[:].rearrange("p (h e) -> p h e", h=H)
            eng = VEV_SCHED[sc % len(VEV_SCHED)]
            if eng == "a":
                nc.scalar.copy(dst, src)
            else:
                nc.vector.tensor_copy(dst, src)

        # ---------------- attention per head ----------------
        e16p = ctx.enter_context(tc.tile_pool(name="e16", bufs=12))
        sctxp = ctx.enter_context(tc.tile_pool(name="sctx", bufs=1))
        smallp = ctx.enter_context(tc.tile_pool(name="small", bufs=1))
        recipp = ctx.enter_context(tc.tile_pool(name="recip", bufs=12))
        osbp = ctx.enter_context(tc.tile_pool(name="osb", bufs=3))
        # 5th q-chunk = the masked-tail "query" (ctx == mean_k v broadcast)
        sbctx = sctxp.tile([128, 5, D], F16, tag="sbctx", name="sbctx")

        def scores_exp(h, e16):
            hb = 64 * (h % 2)
            hc = h // 2
            for t in range(SC // 2):
                psc = P["ps"].tile([128, 2, SH], FP, tag="ps")
                for j in range(2):
                    kc = 2 * t + j
                    nc.tensor.matmul(
                        psc[:, j, :],
                        KT[hb : hb + 64, hc, kc * 128 : (kc + 1) * 128],
                        QT[hb : hb + 64, hc, :],
                        start=True, stop=True,
                    )
                eng = EXP_SCHED[(h * 4 + t) % len(EXP_SCHED)]
                dst = e16[:, 2 * t : 2 * t + 2, :]
                if eng == "a":
                    nc.scalar.activation(dst, psc[:], AF.Exp, scale=SCORE_SCALE)
                else:
                    e_i16 = dst.bitcast(I16)
                    veng = nc.vector if eng == "d" else nc.gpsimd
                    veng.tensor_scalar(e_i16, psc[:], SCH_A, SCH_B,
                                       ALU.mult, ALU.add)

        def ctx_head(h, e16):
            pctx = P["pc"].tile([128, 4, 65], FP, tag="pc",
                                 padded_shape=[128, 4, 128])
            for qc in range(4):
                for kc in range(SC):
                    nc.tensor.matmul(
                        pctx[:, qc, :],
                        e16[:, kc, qc * 128 : (qc + 1) * 128],
                        Vaug[:, kc, h * 65 : (h + 1) * 65],
                        start=(kc == 0), stop=(kc == SC - 1),
                    )
            recip = recipp.tile([128, 4, 1], FP, tag="recip")
            nc.vector.reciprocal(recip[:], pctx[:, :, 64:65])
            dst = sbctx[:, 0:4, h * 64 : (h + 1) * 64]
            # PSUM-reading tensor_tensor: DVE only (ACT has no TT, Pool no PSUM)
            nc.vector.tensor_tensor(
                dst, pctx[:, :, 0:64], recip[:].broadcast_to([128, 4, 64]),
                ALU.mult,
            )

        # ---------------- tail (masked rows): mean_k v ----------------
        # The x64-scaled mean-v row becomes sbctx q-chunk 4 (broadcast to all
        # 128 partitions); the regular transpose + DR out-proj then computes
        # 128 identical tail rows in one go.
        def tail_a():
            # sum_k v via ones^T @ Vaug on the PE; two groups (partitions 0/32)
            pw = P["px"].tile([128, 512], FP, tag="pxw")
            for sc in range(SC):
                nc.tensor.matmul(pw[0:1, 0:390], ones16[:, 0:1],
                                 Vaug[:, sc, 0:390],
                                 start=(sc == 0), stop=(sc == SC - 1))
            for sc in range(SC):
                nc.tensor.matmul(pw[32:33, 0:390], ones16[:, 0:1],
                                 Vaug[:, sc, 390:780],
                                 start=(sc == 0), stop=(sc == SC - 1))
            vrow = smallp.tile([1, H * 65], FP, tag="vrow", name="vrow")
            nc.scalar.copy(vrow[0:1, 0:390], pw[0:1, 0:390])
            nc.scalar.copy(vrow[0:1, 390:780], pw[32:33, 0:390])
            # strided row read (skip denominator columns); keep the x64 scale
            vsb = smallp.tile([1, D], F16, tag="vsb", name="vsb")
            nc.scalar.mul(
                vsb[:].rearrange("p (h e) -> p h e", h=H),
                vrow[:].rearrange("p (h e) -> p h e", h=H)[:, :, 0:64],
                1.0 / S,
            )
            if with_bias:
                bv_row = smallp.tile([1, D], F16, tag="bv_row")
                nc.vector.tensor_scalar_mul(bv_row[:], bvr_f[:], WS)
                nc.vector.tensor_tensor(vsb[:], vsb[:], bv_row[:], ALU.add)
            nc.gpsimd.partition_broadcast(sbctx[:, 4, :], vsb[0:1, :])

        # ---------------- out projection per q chunk ----------------
        ctxTh = qkp.tile([128, DC, SH + 128], F8, tag="ctxTh")
        ctxTl = qkp.tile([128, DC, SH + 128], F8, tag="ctxTl")

        def out_transpose(qc):
            ptT = P["pT"].tile([128, DC * 128], F16, tag="pxT")
            for c in range(DC):
                nc.tensor.transpose(
                    ptT[:, c * 128 : (c + 1) * 128],
                    sbctx[:, qc, c * 128 : (c + 1) * 128],
                    ident[:],
                )
            srcv = ptT[:].rearrange("p (c s) -> p c s", c=DC)
            dsth = ctxTh[:, :, qc * 128 : (qc + 1) * 128]
            dstl = ctxTl[:, :, qc * 128 : (qc + 1) * 128]
            nc.scalar.copy(dsth, srcv)
            # PSUM-reading tensor_tensor: DVE only
            nc.vector.tensor_tensor(dstl, srcv, dsth, ALU.subtract)

        def out_chunk(qc, split_last=False):
            po = P["po"].tile([128, D], FP, tag="ppo",
                              padded_shape=[128, 1024])
            for n0, nw in ((0, 512), (512, 256)):
                for kp in range(DC // 2):
                    _mm_hilo(
                        nc, po[:, n0 : n0 + nw],
                        ctxTh[:, 2 * kp : 2 * kp + 2, qc * 128 : (qc + 1) * 128],
                        ctxTl[:, 2 * kp : 2 * kp + 2, qc * 128 : (qc + 1) * 128],
                        wt["woh"][:, 2 * kp : 2 * kp + 2, n0 : n0 + nw],
                        wt["wol"][:, 2 * kp : 2 * kp + 2, n0 : n0 + nw],
                        first=(kp == 0),
                        last=(not with_bias and kp == DC // 2 - 1),
                    )
                if with_bias:
                    nc.tensor.matmul(
                        po[:, n0 : n0 + nw], ones1[0:1, 0:128],
                        bo_row[0:1, n0 : n0 + nw], start=False, stop=True,
                    )
            osb = osbp.tile([128, D], F16, tag="osb")
            engs = [nc.sync, nc.scalar, nc.gpsimd]
            if qc == 4:  # masked tail: 128 identical rows -> all 4 tail chunks
                nc.vector.tensor_scalar_mul(osb[:], po[:], OUT_DESCALE)
                for i, sc in enumerate(range(SH // 128, SC)):
                    engs[i % 3].dma_start(
                        out=out[sc * 128 : (sc + 1) * 128, :], in_=osb[:]
                    )
                return
            if split_last:
                # split evac+DMA in half so the last DMA starts sooner
                nc.vector.tensor_scalar_mul(osb[:, 0:384], po[:, 0:384],
                                            OUT_DESCALE)
                nc.sync.dma_start(out=out[qc * 128 : (qc + 1) * 128, 0:384],
                                  in_=osb[:, 0:384])
                nc.scalar.mul(osb[:, 384:768], po[:, 384:768], OUT_DESCALE)
                nc.scalar.dma_start(out=out[qc * 128 : (qc + 1) * 128, 384:768],
                                    in_=osb[:, 384:768])
                return
            if qc % 2 == 1:
                nc.scalar.mul(osb[:], po[:], OUT_DESCALE)
            else:
                nc.vector.tensor_scalar_mul(osb[:], po[:], OUT_DESCALE)
            engs[qc % 3].dma_start(
                out=out[qc * 128 : (qc + 1) * 128, :], in_=osb[:]
            )

        # ---------------- schedule ----------------
        e16s = {}

        def emit_scores(h):
            e16s[h] = e16p.tile([128, SC, SH], F16, tag="e16", name=f"e16_{h}")
            scores_exp(h, e16s[h])

        with (
            tc.tile_pool(name="ppq", bufs=3, space="PSUM") as ppq_pool,
            tc.tile_pool(name="ppk", bufs=3, space="PSUM") as ppk_pool,
        ):
            P["ppq"], P["ppk"] = ppq_pool, ppk_pool
            mark("qt")
            # first chunk split at 256 so it can start on the first x piece
            qt_chunk(0, ((0, 256), (256, 256)))
            qt_chunk(1, ((0, 256), (256, 256)))
            for c in range(2, DC):
                qt_chunk(c, ((0, SH),))
            mark("kt0")
            for c in range(DC):
                kt_chunk(c, 0)
            mark("kt1")
            for c in range(DC):
                kt_chunk(c, 1)
        with tc.tile_pool(name="ps", bufs=3, space="PSUM") as ps_pool:
            P["ps"] = ps_pool
            # V chunks interleaved with early heads' scores+exp
            with tc.tile_pool(name="ppv", bufs=1, space="PSUM") as ppv_pool:
                P["ppv"] = ppv_pool
                mark("v+scores")
                for sc in range(SC):
                    v_chunk(sc)
                    if sc < 6:
                        emit_scores(sc)
            # tail pieces interleaved with the last pre-head score emissions
            with tc.tile_pool(name="px", bufs=1, space="PSUM") as px_pool:
                P["px"] = px_pool
                mark("tail_a")
                tail_a()
                emit_scores(6)
                emit_scores(7)
                emit_scores(8)
            with tc.tile_pool(name="pc", bufs=2, space="PSUM") as pc_pool:
                P["pc"] = pc_pool
                mark("heads")
                for h in range(H):
                    mark(f"head{h}")
                    ctx_head(h, e16s[h])
                    del e16s[h]
                    if h + 9 < H:
                        emit_scores(h + 9)

        with (
            tc.tile_pool(name="po", bufs=2, space="PSUM") as po_pool,
            tc.tile_pool(name="pT", bufs=2, space="PSUM") as pT_pool,
        ):
            P["po"], P["pT"] = po_pool, pT_pool
            mark("out")
            # tail chunk first: its 4 big DMAs overlap the active-row compute
            for qc in (4, 0, 1, 2, 3):
                out_transpose(qc)
                out_chunk(qc, split_last=(qc == 3))


def build_nc(with_bias=False):
    nc = bacc.Bacc("TRN2", target_bir_lowering=False, debug=False, num_devices=NCORES)
    t_in = {}
    t_in["xh8"] = nc.dram_tensor("xh8", [128, DC, S], F8, kind="ExternalInput").ap()
    t_in["xl8"] = nc.dram_tensor("xl8", [128, DC, S], F8, kind="ExternalInput").ap()
    for nm in ("wqh", "wql", "wkh", "wkl"):
        t_in[nm] = nc.dram_tensor(nm, [128, DC, DC, 128], F8,
                                  kind="ExternalInput").ap()
    for nm in ("wvh", "wvl", "woh", "wol"):
        t_in[nm] = nc.dram_tensor(nm, [128, DC, D], F8, kind="ExternalInput").ap()
    if with_bias:
        for nm in ("bq", "bk", "bv", "bo"):
            t_in[nm] = nc.dram_tensor(nm, [D], FP, kind="ExternalInput").ap()
    out = nc.dram_tensor("out", [S, D], F16, kind="ExternalOutput").ap()
    with tile.TileContext(nc) as tc:
        _body(tc, out, t_in, with_bias=with_bias)
    nc.compile()
    return nc


def _hilo8(a):
    import ml_dtypes

    hi = a.astype(ml_dtypes.float8_e4m3)
    lo = (a - hi.astype(np.float32)).astype(ml_dtypes.float8_e4m3)
    return hi, lo


def prep_weights(Wq, Wk, Wv, Wo):
    """Host-side: scale, hi/lo split, and relayout of weights."""
    def lay(w):  # [D_in, D_out] -> [128, DC_in, D_out], partition = d_in % 128
        return np.ascontiguousarray(w.reshape(DC, 128, D).transpose(1, 0, 2))

    def laycc(w):  # [D_in, D_out] -> [128, DC_out, DC_in, 128] c-chunk-major
        return np.ascontiguousarray(
            w.reshape(DC, 128, DC, 128).transpose(1, 2, 0, 3))

    outp = {}
    for nm, w, f in (("wq", Wq, laycc), ("wk", Wk, laycc),
                     ("wv", Wv, lay), ("wo", Wo, lay)):
        hi, lo = _hilo8(f(np.asarray(w, np.float32) * WS))
        outp[nm + "h"], outp[nm + "l"] = hi, lo
    return outp


def prep_x(x1):
    """Host-side: [S, D] -> fp8 hi/lo xT [128, DC, S]."""
    xT = np.ascontiguousarray(np.asarray(x1, np.float32).T.reshape(DC, 128, S)
                              .transpose(1, 0, 2))
    hi, lo = _hilo8(xT)
    return hi, lo


def kernel(hidden_states, Wq, bq, Wk, bk, Wv, bv, Wo, bo, _trace=False):
    x = np.asarray(hidden_states, np.float32)
    wshared = prep_weights(Wq, Wk, Wv, Wo)
    biases = {nm: np.ascontiguousarray(np.asarray(v, np.float32))
              for nm, v in (("bq", bq), ("bk", bk), ("bv", bv), ("bo", bo))}
    with_bias = any(np.any(v) for v in biases.values())
    nc = build_nc(with_bias=with_bias)
    in_maps = []
    for i in range(NCORES):
        xh, xl = prep_x(x[i])
        m = {"xh8": xh, "xl8": xl, **wshared}
        if with_bias:
            m.update(biases)
        in_maps.append(m)
    res = run_bass_kernel_spmd(nc, in_maps, core_ids=list(range(NCORES)),
                               trace=_trace)
    out = np.stack([res.results[i]["out"] for i in range(NCORES)],
                   axis=0).astype(np.float32)
    if _trace:
        kernel.last_results = res
    return out


if __name__ == "__main__":
    rng = np.random.default_rng(0)
    ins = {
        "hidden_states": rng.standard_normal((B, S, D), dtype=np.float32),
        **{w: (rng.standard_normal((D, D)) / np.sqrt(D)).astype(np.float32)
           for w in ("Wq", "Wk", "Wv", "Wo")},
        **{b: np.zeros(D, np.float32) for b in ("bq", "bk", "bv", "bo")},
    }
    o = kernel(**ins)
    print("kernel ran, out shape", o.shape)


# revision 61
# speedup vs baseline: 1.0719x; 1.0719x over previous
"""Trainium2 Bass kernel for nn_Attention_28020366639391 (sparse attention), v3.

Math (per batch element b, reference semantics):
    q/k/v = x @ W{q,k,v} + b, 12 heads of 64; scores = q k^T / 8
    rows >= 512 zeroed pre-softmax -> those ctx rows = mean_k(v)
    out = concat_heads(ctx) @ Wo + bo

Sharding: data-parallel on batch, 8 elements -> 8 cores, no collectives.

Per-core dataflow (v3 = v2 + latency/overhead work):
  Host prep: xT in fp8 hi/lo as two contiguous seq-half tiles ([128, 6, 512]
  each; contiguous DMAs move ~2x faster than strided), weights pre-scaled by
  64 then split fp8 hi/lo; Wq/Wk relaid c-chunk-major
  ([128, 6(d_out chunk), 6(d_in chunk), 128]) so startup DMAs stream
  per-output-chunk pieces; Wo in fp8 hi/lo (also x64).
  - QKV projections: fp8 DoubleRow, 3 products (hh + hl + lh) ~ bf16-accurate
    at 0.5 cycles/row. QT/KT evac to fp16 [d_out, q]; V evac to fp16
    Vaug [keys, head*65] with a 1.0 column per head (softmax denominator;
    the x64 V scale then rides into sbctx, cancelled at the final evac).
  - scores (per head, per 256-key pair): fp16 matmuls into a paired PSUM
    tile [128, 2, 512]; ONE exp op per pair ([128, 1024]) via ACT (exact)
    or DVE (Schraudolph int16->fp16 bit trick). Score pairs are emitted as
    soon as their KT chunk exists, filling the PE through the DMA-bound
    first ~20us (the HBM transfer channel serializes globally at ~180GB/s,
    and ALL fp8 anywhere on the softmax path fails the 2e-2 gate: softmax
    weight wobble passes through at full relative magnitude, so scores/e/v
    must stay fp16 and the PE floor is ~73us).
  - ctx (layout B, kc-major): out[q,65] += e_slice^T @ Vaug_h per key chunk.
  - normalize fused into PSUM evac: ctx * recip(denom) -> sbctx [q, d] fp16
    (values x64 because the denom column is 1.0 while v carries x64).
  - PE-transpose sbctx -> ptT; evac splits into ctxT fp8 hi/lo; out-proj is
    fp8 DoubleRow 3-product vs Wo8 hi/lo; final evac scales by 1/4096.
  - tail rows 512:1024: mean_k v via a ones^T @ Vaug PE reduction becomes
    sbctx q-chunk 4 (partition broadcast), so the regular transpose +
    out-proj machinery computes 128 identical tail rows (hi-only product;
    tail rows are tiny vs the global max so 1.8% there is harmless).
  - p-state: the PE drops to 1.2GHz for ~3us after any idle gap, so warmup
    matmuls on a zero tile keep it ramped through the startup DMA wait,
    and the schedule leans on deep PSUM pool pipelines (ps=3, pc=4, pT=4)
    to avoid mid-kernel gaps. PSUM pool creation order is chosen so bank
    reuse across regions isn't gated on slow evacuations.
  - drain: the last out chunk evacuates+DMAs its first 512 columns while
    the last 256-wide PSUM group is still accumulating.
  DMAs are spread across the SP/ACT/Pool DGE queues, demand-ordered; all
  output DMAs go on the SP queue (compute-engine HWDGE queues head-of-line
  block their own evac ops).
"""

import numpy as np

import concourse.bass as bass
import concourse.mybir as mybir
import concourse.tile as tile
from concourse import bacc
from concourse.bass_utils import run_bass_kernel_spmd
from concourse.masks import make_identity

B, S, D, H, DH = 8, 1024, 768, 12, 64
SH = 512            # active query rows
DC = D // 128       # 6 chunks of model dim
SC = S // 128       # 8 chunks of sequence dim
NCORES = 8
WS = 64.0           # host-side weight scaling (fp8 lo-residual range fix)
FP = mybir.dt.float32
F16 = mybir.dt.float16
F8 = mybir.dt.float8e4
I16 = mybir.dt.int16
AF = mybir.ActivationFunctionType
ALU = mybir.AluOpType
DR = mybir.MatmulPerfMode.DoubleRow

LOG2E = 1.4426950408889634
SCORE_SCALE = 0.125 / (WS * WS)      # folded 1/64^2 weight scaling
SCH_A = SCORE_SCALE * LOG2E * 1024.0 # schraudolph int16 multiplier
SCH_B = 15.0 * 1024.0 - 44.0         # schraudolph bias (tuned C=44, floor conv)
OUT_DESCALE = 1.0 / (WS * WS)        # ctx(x64) @ Wo(x64) -> /4096 at evac

# exp engine per (head, kc-pair): a=ACT(exact exp), d=DVE (schraudolph)
EXP_SCHED = "aadadaadadad"
# V-evac engine per sequence chunk
VEV_SCHED = "adadadad"


def _mm_hilo(nc, out, lhs_hi, lhs_lo, rhs_hi, rhs_lo, first, last,
             lo_last="lhs"):
    """3-product hi/lo fp8 DoubleRow accumulation into one PSUM group.

    lo_last picks which lo-product goes last, so the operand that arrives
    last on the serial DMA stream doesn't stall the group start."""
    nc.tensor.matmul(out, lhs_hi, rhs_hi, start=first, stop=False, perf_mode=DR)
    if lo_last == "lhs":
        nc.tensor.matmul(out, lhs_hi, rhs_lo, start=False, stop=False,
                         perf_mode=DR)
        nc.tensor.matmul(out, lhs_lo, rhs_hi, start=False, stop=last,
                         perf_mode=DR)
    else:
        nc.tensor.matmul(out, lhs_lo, rhs_hi, start=False, stop=False,
                         perf_mode=DR)
        nc.tensor.matmul(out, lhs_hi, rhs_lo, start=False, stop=last,
                         perf_mode=DR)


PHASES = []


def _body(tc, out, t_in, with_bias=False):
    nc = tc.nc
    from contextlib import ExitStack

    def mark(nm):
        PHASES.append((nm, len(list(nc.all_instructions()))))

    with ExitStack() as ctx:
        ctx.enter_context(
            nc.allow_low_precision(reason="fp8 hi/lo + fp16 pipeline by design")
        )
        constp = ctx.enter_context(tc.tile_pool(name="const", bufs=1))
        wp = ctx.enter_context(tc.tile_pool(name="wp", bufs=1))
        qkp = ctx.enter_context(tc.tile_pool(name="qk", bufs=1))
        P = {}  # phase-scoped PSUM pools, resolved at call time

        # ---------------- DMA inputs (parallel queues, ordered by need) ------
        # x as two seq-half tiles so each input DMA is fully contiguous
        # (contiguous transfers move ~2x faster than strided slices).
        wt = {}
        for nm in ("xh8A", "xl8A", "xh8B", "xl8B"):
            wt[nm] = wp.tile([128, DC, SH], F8, tag=nm, name=nm)
        for nm in ("wqh", "wql", "wkh", "wkl"):
            # c-chunk-major relayout: [128, c(d_out/128), kp(d_in/128), 128]
            wt[nm] = wp.tile([128, DC, DC, 128], F8, tag=nm, name=nm)
        for nm in ("wvh", "wvl", "woh", "wol"):
            wt[nm] = wp.tile([128, DC, D], F8, tag=nm, name=nm)
        # ---------------- constants (BEFORE dma issue: the Pool engine also
        # runs SWDGE desc-gen for its DMA queue, which must not delay the
        # identity needed by the p-state warmup transposes) ----------------
        ident = constp.tile([128, 128], F16, tag="ident")
        identf = constp.tile([128, 128], FP, tag="identf")
        make_identity(nc, identf[:])
        nc.vector.tensor_copy(ident[:], identf[:])
        ones16 = constp.tile([128, 1], F16, tag="ones16")
        nc.gpsimd.memset(ones16[:], 1.0)

        # demand-ordered DMA list; queues: SP / ACT HWDGE + Pool SWDGE run
        # their desc-gens in parallel, transfers share the HBM channel.
        dmas = [
            (wt["xh8A"][:], t_in["xh8"][:, :, 0:512]),
            (wt["wqh"][:, 0:2], t_in["wqh"][:, 0:2]),
            (wt["xl8A"][:], t_in["xl8"][:, :, 0:512]),
            (wt["wql"][:, 0:2], t_in["wql"][:, 0:2]),
            (wt["wqh"][:, 2:6], t_in["wqh"][:, 2:6]),
            (wt["wkh"][:, 0:3], t_in["wkh"][:, 0:3]),
            (wt["wql"][:, 2:6], t_in["wql"][:, 2:6]),
            (wt["wkl"][:, 0:3], t_in["wkl"][:, 0:3]),
            (wt["wkh"][:, 3:6], t_in["wkh"][:, 3:6]),
            (wt["wkl"][:, 3:6], t_in["wkl"][:, 3:6]),
            (wt["xh8B"][:], t_in["xh8"][:, :, 512:1024]),
            (wt["xl8B"][:], t_in["xl8"][:, :, 512:1024]),
            (wt["wvh"][:], t_in["wvh"][...]),
            (wt["wvl"][:], t_in["wvl"][...]),
            (wt["woh"][:], t_in["woh"][...]),
            (wt["wol"][:], t_in["wol"][...]),
        ]
        qs = [nc.sync, nc.scalar, nc.gpsimd]
        for i, (dst, src_) in enumerate(dmas):
            qs[i % 3].dma_start(out=dst, in_=src_)

        def xslice(kp, n0, nw):
            """x moving operand [128, 2, nw] for seq window [n0, n0+nw)."""
            assert n0 // SH == (n0 + nw - 1) // SH
            half = "A" if n0 < SH else "B"
            o = n0 % SH
            return (wt["xh8" + half][:, 2 * kp : 2 * kp + 2, o : o + nw],
                    wt["xl8" + half][:, 2 * kp : 2 * kp + 2, o : o + nw])

        wud = constp.tile([128, 128], F16, tag="wud")
        nc.vector.memset(wud[:], 0.0)  # DVE: ready ~immediately at t=0

        def warmup(n):
            # keep the PE p-state ramped through known idle/DMA-wait windows;
            # plain matmuls on a zero const (no identity dependency), into a
            # reused qkt PSUM buf to avoid a dedicated bank
            pwu = P["ppq"].tile([128, SH], FP, tag="pp")
            for _ in range(n):
                nc.tensor.matmul(pwu[:, 0:128], wud[:], wud[:],
                                 start=True, stop=True)

        QT = qkp.tile([128, DC, SH], F16, tag="QT")
        KT = qkp.tile([128, DC, S], F16, tag="KT")
        Vaug = qkp.tile([128, SC, H * 65], F16, tag="Vaug")
        # denominator columns: 1.0 (v carries x64; net sbctx = 64*ctx_true)
        nc.gpsimd.memset(
            Vaug[:].rearrange("p s (h e) -> p s h e", h=H)[:, :, :, 64:65], 1.0
        )

        if with_bias:
            b_sb = {}
            for nm in ("bq", "bk", "bv"):
                t = constp.tile([128, DC], FP, tag=f"b_{nm}", name=f"b_{nm}")
                for c in range(DC):
                    nc.sync.dma_start(
                        out=t[:, c : c + 1], in_=t_in[nm][c * 128 : (c + 1) * 128, None]
                    )
                b_sb[nm] = t
            for nm in ("bq", "bk", "bv"):  # match the x64-scaled q/k/v outputs
                nc.vector.tensor_scalar_mul(b_sb[nm][:], b_sb[nm][:], WS)
            bo_row = constp.tile([1, D], F16, tag="bo_row")
            bo_f = constp.tile([1, D], FP, tag="bo_f")
            bvr_f = constp.tile([1, D], FP, tag="bvr_f")
            nc.sync.dma_start(out=bvr_f[:], in_=t_in["bv"][None, :])
            nc.sync.dma_start(out=bo_f[:], in_=t_in["bo"][None, :])
            # po carries x4096; pre-scale bo to survive the 1/4096 evac
            nc.vector.tensor_scalar_mul(bo_f[:], bo_f[:], WS * WS)
            nc.vector.tensor_copy(bo_row[:], bo_f[:])
            ones1 = constp.tile([1, 128], F16, tag="ones1")
            nc.vector.memset(ones1[:], 1.0)

        # ---------------- QKV projections (fp8 DoubleRow hi/lo) --------------
        def qt_chunk(c, splits):
            pq = P["ppq"].tile([128, SH], FP, tag="pp")
            for n0, nw in splits:
                for kp in range(DC // 2):
                    xh, xl = xslice(kp, n0, nw)
                    _mm_hilo(
                        nc, pq[:, n0 : n0 + nw],
                        wt["wqh"][:, c, 2 * kp : 2 * kp + 2, :],
                        wt["wql"][:, c, 2 * kp : 2 * kp + 2, :],
                        xh, xl,
                        first=(kp == 0), last=(kp == DC // 2 - 1),
                    )
            if with_bias:
                nc.scalar.activation(
                    QT[:, c, :], pq[:], AF.Identity, bias=b_sb["bq"][:, c : c + 1]
                )
            elif c % 2 == 0:
                nc.scalar.copy(QT[:, c, :], pq[:])
            else:
                nc.vector.tensor_copy(QT[:, c, :], pq[:])

        def kt_chunk(c, sg):
            pk = P["ppk"].tile([128, 512], FP, tag="pp")
            for kp in range(DC // 2):
                xh, xl = xslice(kp, sg * 512, 512)
                _mm_hilo(
                    nc, pk[:],
                    wt["wkh"][:, c, 2 * kp : 2 * kp + 2, :],
                    wt["wkl"][:, c, 2 * kp : 2 * kp + 2, :],
                    xh, xl,
                    first=(kp == 0), last=(kp == DC // 2 - 1),
                )
            dst = KT[:, c, sg * 512 : sg * 512 + 512]
            if with_bias:
                nc.vector.tensor_scalar(dst, pk[:], b_sb["bk"][:, c : c + 1],
                                        None, ALU.add)
            elif (c + sg) % 2 == 0:
                nc.vector.tensor_copy(dst, pk[:])
            else:
                nc.scalar.copy(dst, pk[:])

        def v_chunk(sc):
            pv = P["ppv"].tile([128, D], FP, tag="ppv",
                               padded_shape=[128, 1024])
            for n0, nw in ((0, 512), (512, 256)):
                for kp in range(DC // 2):
                    xh, xl = xslice(kp, sc * 128, 128)
                    _mm_hilo(
                        nc, pv[:, n0 : n0 + nw],
                        xh, xl,
                        wt["wvh"][:, 2 * kp : 2 * kp + 2, n0 : n0 + nw],
                        wt["wvl"][:, 2 * kp : 2 * kp + 2, n0 : n0 + nw],
                        first=(kp == 0), last=(kp == DC // 2 - 1),
                        lo_last="rhs",
                    )
            dst = Vaug[:].rearrange("p s (h e) -> p s h e", h=H)[:, sc, :, 0:64]
            src = pv[:].rearrange("p (h e) -> p h e", h=H)
            eng = VEV_SCHED[sc % len(VEV_SCHED)]
            if eng == "a":
                nc.scalar.copy(dst, src)
            else:
                nc.vector.tensor_copy(dst, src)

        # ---------------- attention per head ----------------
        e16p = ctx.enter_context(tc.tile_pool(name="e16", bufs=12))
        sctxp = ctx.enter_context(tc.tile_pool(name="sctx", bufs=1))
        smallp = ctx.enter_context(tc.tile_pool(name="small", bufs=1))
        recipp = ctx.enter_context(tc.tile_pool(name="recip", bufs=12))
        osbp = ctx.enter_context(tc.tile_pool(name="osb", bufs=3))
        # 5th q-chunk = the masked-tail "query" (ctx == mean_k v broadcast)
        sbctx = sctxp.tile([128, 5, D], F16, tag="sbctx", name="sbctx")

        def scores_pair(h, e16, t):
            hb = 64 * (h % 2)
            hc = h // 2
            psc = P["ps"].tile([128, 2, SH], FP, tag="ps")
            for j in range(2):
                kc = 2 * t + j
                nc.tensor.matmul(
                    psc[:, j, :],
                    KT[hb : hb + 64, hc, kc * 128 : (kc + 1) * 128],
                    QT[hb : hb + 64, hc, :],
                    start=True, stop=True,
                )
            eng = EXP_SCHED[(h + t) % len(EXP_SCHED)]
            dst = e16[:, 2 * t : 2 * t + 2, :]
            if eng == "a":
                nc.scalar.activation(dst, psc[:], AF.Exp, scale=SCORE_SCALE)
            else:
                e_i16 = dst.bitcast(I16)
                nc.vector.tensor_scalar(e_i16, psc[:], SCH_A, SCH_B,
                                        ALU.mult, ALU.add)

        def ctx_head(h, e16):
            pctx = P["pc"].tile([128, 4, 65], FP, tag="pc",
                                 padded_shape=[128, 4, 128])
            for qc in range(4):
                for kc in range(SC):
                    nc.tensor.matmul(
                        pctx[:, qc, :],
                        e16[:, kc, qc * 128 : (qc + 1) * 128],
                        Vaug[:, kc, h * 65 : (h + 1) * 65],
                        start=(kc == 0), stop=(kc == SC - 1),
                    )
            recip = recipp.tile([128, 4, 1], FP, tag="recip")
            nc.vector.reciprocal(recip[:], pctx[:, :, 64:65])
            dst = sbctx[:, 0:4, h * 64 : (h + 1) * 64]
            # PSUM-reading tensor_tensor: DVE only (ACT has no TT, Pool no PSUM)
            nc.vector.tensor_tensor(
                dst, pctx[:, :, 0:64], recip[:].broadcast_to([128, 4, 64]),
                ALU.mult,
            )

        # ---------------- tail (masked rows): mean_k v ----------------
        # The x64-scaled mean-v row becomes sbctx q-chunk 4 (broadcast to all
        # 128 partitions); the regular transpose + DR out-proj then computes
        # 128 identical tail rows in one go.
        def tail_a():
            # sum_k v via ones^T @ Vaug on the PE; two groups (partitions 0/32)
            pw = P["px"].tile([128, 512], FP, tag="pxw")
            for sc in range(SC):
                nc.tensor.matmul(pw[0:1, 0:390], ones16[:, 0:1],
                                 Vaug[:, sc, 0:390],
                                 start=(sc == 0), stop=(sc == SC - 1))
            for sc in range(SC):
                nc.tensor.matmul(pw[32:33, 0:390], ones16[:, 0:1],
                                 Vaug[:, sc, 390:780],
                                 start=(sc == 0), stop=(sc == SC - 1))
            vrow = smallp.tile([1, H * 65], FP, tag="vrow", name="vrow")
            nc.scalar.copy(vrow[0:1, 0:390], pw[0:1, 0:390])
            nc.scalar.copy(vrow[0:1, 390:780], pw[32:33, 0:390])
            # strided row read (skip denominator columns); keep the x64 scale
            vsb = smallp.tile([1, D], F16, tag="vsb", name="vsb")
            nc.scalar.mul(
                vsb[:].rearrange("p (h e) -> p h e", h=H),
                vrow[:].rearrange("p (h e) -> p h e", h=H)[:, :, 0:64],
                1.0 / S,
            )
            if with_bias:
                bv_row = smallp.tile([1, D], F16, tag="bv_row")
                nc.vector.tensor_scalar_mul(bv_row[:], bvr_f[:], WS)
                nc.vector.tensor_tensor(vsb[:], vsb[:], bv_row[:], ALU.add)
            nc.gpsimd.partition_broadcast(sbctx[:, 4, :], vsb[0:1, :])

        # ---------------- out projection per q chunk ----------------
        ctxTh = qkp.tile([128, DC, SH + 128], F8, tag="ctxTh")
        ctxTl = qkp.tile([128, DC, SH + 128], F8, tag="ctxTl")

        def tr_evac(qc, c):
            # transpose + hi/lo evac of one (q-chunk, d-chunk) tile; d-chunk c
            # only needs heads 2c/2c+1, so these run DURING the heads region
            ptT = P["pT"].tile([128, 128], F16, tag="pxT")
            nc.tensor.transpose(ptT[:], sbctx[:, qc, c * 128 : (c + 1) * 128],
                                ident[:])
            dsth = ctxTh[:, c, qc * 128 : (qc + 1) * 128]
            dstl = ctxTl[:, c, qc * 128 : (qc + 1) * 128]
            nc.scalar.copy(dsth, ptT[:])
            if qc != 4:  # tail rows tolerate the hi-only product
                # PSUM-reading tensor_tensor: DVE only
                nc.vector.tensor_tensor(dstl, ptT[:], dsth, ALU.subtract)

        def out_chunk(qc, split_last=False):
            po = P["po"].tile([128, D], FP, tag="ppo",
                              padded_shape=[128, 1024])
            osb = osbp.tile([128, D], F16, tag="osb")
            for n0, nw in ((0, 512), (512, 256)):
                for kp in range(DC // 2):
                    if qc == 4:  # tail: hi-only product is accurate enough
                        nc.tensor.matmul(
                            po[:, n0 : n0 + nw],
                            ctxTh[:, 2 * kp : 2 * kp + 2,
                                  qc * 128 : (qc + 1) * 128],
                            wt["woh"][:, 2 * kp : 2 * kp + 2, n0 : n0 + nw],
                            start=(kp == 0),
                            stop=(not with_bias and kp == DC // 2 - 1),
                            perf_mode=DR,
                        )
                        continue
                    _mm_hilo(
                        nc, po[:, n0 : n0 + nw],
                        ctxTh[:, 2 * kp : 2 * kp + 2, qc * 128 : (qc + 1) * 128],
                        ctxTl[:, 2 * kp : 2 * kp + 2, qc * 128 : (qc + 1) * 128],
                        wt["woh"][:, 2 * kp : 2 * kp + 2, n0 : n0 + nw],
                        wt["wol"][:, 2 * kp : 2 * kp + 2, n0 : n0 + nw],
                        first=(kp == 0),
                        last=(not with_bias and kp == DC // 2 - 1),
                    )
                if with_bias:
                    nc.tensor.matmul(
                        po[:, n0 : n0 + nw], ones1[0:1, 0:128],
                        bo_row[0:1, n0 : n0 + nw], start=False, stop=True,
                    )
            engs = [nc.sync, nc.scalar, nc.gpsimd]
            if qc == 4:  # masked tail: 128 identical rows -> all 4 tail chunks
                nc.vector.tensor_scalar_mul(osb[:], po[:], OUT_DESCALE)
                for i, sc in enumerate(range(SH // 128, SC)):
                    (nc.sync if i % 2 == 0 else nc.scalar).dma_start(
                        out=out[sc * 128 : (sc + 1) * 128, :], in_=osb[:]
                    )
                return
            if split_last:
                # split evac+DMA in half so the last DMA starts sooner
                nc.vector.tensor_scalar_mul(osb[:, 0:384], po[:, 0:384],
                                            OUT_DESCALE)
                nc.sync.dma_start(out=out[qc * 128 : (qc + 1) * 128, 0:384],
                                  in_=osb[:, 0:384])
                nc.scalar.mul(osb[:, 384:768], po[:, 384:768], OUT_DESCALE)
                nc.scalar.dma_start(out=out[qc * 128 : (qc + 1) * 128, 384:768],
                                    in_=osb[:, 384:768])
                return
            if qc % 2 == 1:
                nc.scalar.mul(osb[:], po[:], OUT_DESCALE)
            else:
                nc.vector.tensor_scalar_mul(osb[:], po[:], OUT_DESCALE)
            (nc.sync if qc % 2 == 0 else nc.scalar).dma_start(
                out=out[qc * 128 : (qc + 1) * 128, :], in_=osb[:]
            )

        # ---------------- schedule ----------------
        e16s = {}

        def stile(h):
            if h not in e16s:
                e16s[h] = e16p.tile([128, SC, SH], F16, tag="e16",
                                    name=f"e16_{h}")
            return e16s[h]

        # Region 1: QT/KT projections with ALL score pairs interleaved.
        # The first ~19us are DMA-throughput-bound (transfers serialize), so
        # the scores (which only need QT + the KT chunks already computed)
        # fill the PE while weights stream in.
        with (
            tc.tile_pool(name="ppqk", bufs=2, space="PSUM") as ppqk_pool,
            tc.tile_pool(name="ps", bufs=3, space="PSUM") as ps_pool,
        ):
            P["ppq"] = P["ppk"] = ppqk_pool
            P["ps"] = ps_pool
            mark("qt")
            warmup(46)  # PE busy+ramped while the first DMAs land
            qt_chunk(0, ((0, 256), (256, 256)))
            qt_chunk(1, ((0, 256), (256, 256)))
            for c in range(2, DC):
                qt_chunk(c, ((0, SH),))
            # each KT chunk enables 2 heads' score pairs; keep bursts at 2
            # pairs so the exp engines don't fall behind the PE
            mark("kt0+s")
            for c in range(DC):
                kt_chunk(c, 0)
                scores_pair(2 * c, stile(2 * c), 0)
                scores_pair(2 * c + 1, stile(2 * c + 1), 0)
            mark("s_t1")
            for h in range(H):
                scores_pair(h, stile(h), 1)
            mark("kt1+s")
            for c in range(DC):
                kt_chunk(c, 1)
                scores_pair(2 * c, stile(2 * c), 2)
                scores_pair(2 * c + 1, stile(2 * c + 1), 2)

        # Region 2: V projections with the last score sweep interleaved
        # (2 pairs after each of v2..v7 keeps both exp engines fed while the
        # v evacs drain); the tail's ones-reduce then covers the v drain.
        with (
            tc.tile_pool(name="ppv", bufs=2, space="PSUM") as ppv_pool,
            tc.tile_pool(name="ps2", bufs=2, space="PSUM") as ps2_pool,
        ):
            P["ppv"] = ppv_pool
            P["ps"] = ps2_pool
            mark("v+s_t3")
            for sc in range(SC):
                v_chunk(sc)
                if sc < 6:
                    scores_pair(2 * sc, stile(2 * sc), 3)
                    scores_pair(2 * sc + 1, stile(2 * sc + 1), 3)
        # Region 3a: ctx heads (deep PSUM pipeline keeps the PE dense);
        # the tail mean-v reduction slots in after head 1, and each odd head
        # releases a ctxT d-chunk: transpose + hi/lo evac immediately so the
        # out-projection region has no vector work left but the final evacs.
        with (
            tc.tile_pool(name="pc", bufs=4, space="PSUM") as pc_pool,
            tc.tile_pool(name="pT", bufs=3, space="PSUM") as pT_pool,
            tc.tile_pool(name="px", bufs=1, space="PSUM") as px_pool,
        ):
            P["pc"], P["pT"], P["px"] = pc_pool, pT_pool, px_pool
            mark("heads")
            for h in range(H):
                mark(f"head{h}")
                ctx_head(h, e16s[h])
                del e16s[h]
                if h == 1:
                    mark("tail_a")
                    tail_a()
                    for qc in range(5):
                        tr_evac(qc, 0)
                elif h % 2 == 1:
                    for qc in range(5):
                        tr_evac(qc, h // 2)

        # Region 3b: out projection matmuls + final evac/DMA only.
        with tc.tile_pool(name="po", bufs=2, space="PSUM") as po_pool:
            P["po"] = po_pool
            mark("out")
            out_chunk(4)
            out_chunk(0)
            out_chunk(1)
            out_chunk(2)
            out_chunk(3, split_last=True)


def build_nc(with_bias=False):
    nc = bacc.Bacc("TRN2", target_bir_lowering=False, debug=False, num_devices=NCORES)
    t_in = {}
    t_in["xh8"] = nc.dram_tensor("xh8", [128, DC, S], F8, kind="ExternalInput").ap()
    t_in["xl8"] = nc.dram_tensor("xl8", [128, DC, S], F8, kind="ExternalInput").ap()
    for nm in ("wqh", "wql", "wkh", "wkl"):
        t_in[nm] = nc.dram_tensor(nm, [128, DC, DC, 128], F8,
                                  kind="ExternalInput").ap()
    for nm in ("wvh", "wvl", "woh", "wol"):
        t_in[nm] = nc.dram_tensor(nm, [128, DC, D], F8, kind="ExternalInput").ap()
    if with_bias:
        for nm in ("bq", "bk", "bv", "bo"):
            t_in[nm] = nc.dram_tensor(nm, [D], FP, kind="ExternalInput").ap()
    out = nc.dram_tensor("out", [S, D], F16, kind="ExternalOutput").ap()
    with tile.TileContext(nc) as tc:
        _body(tc, out, t_in, with_bias=with_bias)
    nc.compile()
    return nc


def _hilo8(a):
    import ml_dtypes

    hi = a.astype(ml_dtypes.float8_e4m3)
    lo = (a - hi.astype(np.float32)).astype(ml_dtypes.float8_e4m3)
    return hi, lo


def prep_weights(Wq, Wk, Wv, Wo):
    """Host-side: scale, hi/lo split, and relayout of weights."""
    def lay(w):  # [D_in, D_out] -> [128, DC_in, D_out], partition = d_in % 128
        return np.ascontiguousarray(w.reshape(DC, 128, D).transpose(1, 0, 2))

    def laycc(w):  # [D_in, D_out] -> [128, DC_out, DC_in, 128] c-chunk-major
        return np.ascontiguousarray(
            w.reshape(DC, 128, DC, 128).transpose(1, 2, 0, 3))

    outp = {}
    for nm, w, f in (("wq", Wq, laycc), ("wk", Wk, laycc),
                     ("wv", Wv, lay), ("wo", Wo, lay)):
        hi, lo = _hilo8(f(np.asarray(w, np.float32) * WS))
        outp[nm + "h"], outp[nm + "l"] = hi, lo
    return outp


def prep_x(x1):
    """Host-side: [S, D] -> fp8 hi/lo xT [128, DC, S]."""
    xT = np.ascontiguousarray(np.asarray(x1, np.float32).T.reshape(DC, 128, S)
                              .transpose(1, 0, 2))
    hi, lo = _hilo8(xT)
    return hi, lo


def kernel(hidden_states, Wq, bq, Wk, bk, Wv, bv, Wo, bo, _trace=False):
    x = np.asarray(hidden_states, np.float32)
    wshared = prep_weights(Wq, Wk, Wv, Wo)
    biases = {nm: np.ascontiguousarray(np.asarray(v, np.float32))
              for nm, v in (("bq", bq), ("bk", bk), ("bv", bv), ("bo", bo))}
    with_bias = any(np.any(v) for v in biases.values())
    nc = build_nc(with_bias=with_bias)
    in_maps = []
    for i in range(NCORES):
        xh, xl = prep_x(x[i])
        m = {"xh8": xh, "xl8": xl, **wshared}
        if with_bias:
            m.update(biases)
        in_maps.append(m)
    res = run_bass_kernel_spmd(nc, in_maps, core_ids=list(range(NCORES)),
                               trace=_trace)
    out = np.stack([res.results[i]["out"] for i in range(NCORES)],
                   axis=0).astype(np.float32)
    if _trace:
        kernel.last_results = res
    return out


if __name__ == "__main__":
    rng = np.random.default_rng(0)
    ins = {
        "hidden_states": rng.standard_normal((B, S, D), dtype=np.float32),
        **{w: (rng.standard_normal((D, D)) / np.sqrt(D)).astype(np.float32)
           for w in ("Wq", "Wk", "Wv", "Wo")},
        **{b: np.zeros(D, np.float32) for b in ("bq", "bk", "bv", "bo")},
    }
    o = kernel(**ins)
    print("kernel ran, out shape", o.shape)
